# revision 1
# baseline (speedup 1.0000x reference)
"""MoE FFN (top-2 of 8 experts) Trainium2 kernel.

Strategy (expert-parallel across 8 NeuronCores):
  - Host computes the (tiny) router: logits = x@Wg, softmax, top-2,
    renormalized combine weights.  Tokens are gathered per expert on the
    host ("all-to-all dispatch" done at sharding time), transposed to
    [H, C] so both FFN GEMMs run with natural weight layouts on device.
  - Core e runs the FFN for expert e over its C_pad gathered tokens,
    F-quarter by F-quarter (quarter weights stream through SBUF,
    double-buffered; chunks of <=512 tokens bound PSUM/SBUF usage):
        hT = gelu_tanh(W1.T-tiles @ xT)        # [Fq, C] per quarter
        Y_fb = hT-tiles.T @ W2_fb              # [C, H] partial per quarter
    Partials land in per-quarter DRAM regions; the host sums them
    (cheaper than device-side DRAM read-back accumulation).
  - Host applies combine weights + b2 and scatter-adds back ("combine").

  All matmuls use float32r (full-rate fp32 tensor-engine mode, fp32
  storage, fp32 PSUM accumulation).

The kernel is compiled once per (C_pad, chunk-structure, biases-zero)
configuration and cached in-process.
"""

import os
import sys
import numpy as np

for _p in ("/opt/trn_rl_repo", "/root/.axon_site/_ro/trn_rl_repo"):
    if _p not in sys.path and os.path.isdir(_p):
        sys.path.append(_p)

import concourse.bacc as bacc  # noqa: E402
import concourse.tile as tile  # noqa: E402
from concourse import mybir  # noqa: E402
from concourse.bass_utils import run_bass_kernel_spmd  # noqa: E402

# Problem shapes (hardcoded per spec)
B, S, H, F, E = 4, 2048, 1024, 4096, 8
T = B * S
TOP_K = 2
N_CORES = 8
P = 128
KH = H // P          # 8  H-contraction subtiles
FT = F // P          # 32 f-tiles total
# F processed in blocks of f-tiles (weights resident per block, streamed
# double-buffered). Equal blocks of 8 measured best: smaller lead blocks
# shorten the head but cost more in GEMM2 accumulation-group overhead.
BLOCKS = (8, 8, 8, 8)
NBLK = len(BLOCKS)
MH = H // P          # 8  output H tiles

F32 = mybir.dt.float32
F32R = mybir.dt.float32r

_CACHE: dict = {}
LAST_RESULT = None  # BassKernelResults of the most recent run (for test.py)


def _chunks_for(c_pad: int) -> tuple:
    """Token chunks: 512s with an optional single 256 tail."""
    out = [512] * (c_pad // 512)
    if c_pad % 512:
        assert c_pad % 512 == 256
        out.append(256)
    return tuple(out)


def _build(c_pad: int, chunks: tuple, use_b1: bool, mm_dt, act_fn=None):
    nc = bacc.Bacc(
        "TRN2",
        target_bir_lowering=False,
        debug=False,
        enable_asserts=False,
        num_devices=N_CORES,
    )

    xd = nc.dram_tensor("xd", [P, KH, c_pad], mm_dt, kind="ExternalInput").ap()
    w1d = nc.dram_tensor("w1d", [P, FT, KH, P], mm_dt, kind="ExternalInput").ap()
    w2d = nc.dram_tensor("w2d", [P, FT, H], mm_dt, kind="ExternalInput").ap()
    if use_b1:
        b1d = nc.dram_tensor("b1d", [P, FT], F32, kind="ExternalInput").ap()
    # per-F-block partial outputs; host sums over the NBLK axis (cheaper than
    # device-side DRAM read-back accumulation, which stalls the PE)
    yd = nc.dram_tensor(
        "yd", [P, NBLK, c_pad // P, H], F32, kind="ExternalOutput"
    ).ap()

    gelu = act_fn or mybir.ActivationFunctionType.Gelu_apprx_tanh

    with tile.TileContext(nc) as tc:
        with (
            tc.tile_pool(name="w1p", bufs=2) as w1p,
            tc.tile_pool(name="w2p", bufs=2) as w2p,
            tc.tile_pool(name="xp", bufs=2) as xp,
            tc.tile_pool(name="hp", bufs=2) as hp,
            tc.tile_pool(name="op", bufs=6) as op,
            tc.tile_pool(name="bp", bufs=1) as bp,
            tc.tile_pool(name="ps1", bufs=3, space="PSUM") as ps1,
            tc.tile_pool(name="ps2", bufs=5, space="PSUM") as ps2,
        ):
            if use_b1:
                b1t = bp.tile([P, FT], F32)
                nc.sync.dma_start(b1t[:], b1d[:])

            fstart = 0
            for bi, fbn in enumerate(BLOCKS):
                # weights stream on the scalar HWDGE ring in ~1MB slices so
                # they never head-of-line-block the x/y traffic (sync ring)
                w1q = w1p.tile([P, fbn, KH, P], mm_dt, tag="w1q", name=f"w1q_{bi}")
                for f in range(fbn):
                    nc.scalar.dma_start(w1q[:, f], w1d[:, fstart + f])
                w2q = w2p.tile([P, fbn, H], mm_dt, tag="w2q", name=f"w2q_{bi}")
                for k2 in range(fbn):
                    nc.scalar.dma_start(w2q[:, k2], w2d[:, fstart + k2])

                coff = 0
                for ci, nt in enumerate(chunks):
                    xt = xp.tile([P, KH, nt], mm_dt, tag="xt")
                    if ci == 0:
                        # halves: GEMM1 k=0..3 can start on the first 1MB
                        nc.sync.dma_start(xt[:, :4], xd[:, :4, coff : coff + nt])
                        nc.sync.dma_start(xt[:, 4:], xd[:, 4:, coff : coff + nt])
                    else:
                        nc.sync.dma_start(xt[:], xd[:, :, coff : coff + nt])

                    # GEMM1: hT[f, :] = gelu(sum_k W1[k, f-tile].T @ xT[k, :])
                    hq = hp.tile([P, fbn, nt], mm_dt, tag="hq", name=f"hq_{bi}")
                    for f in range(fbn):
                        pt1 = ps1.tile([P, nt], F32, tag="pt1")
                        for k in range(KH):
                            nc.tensor.matmul(
                                pt1[:],
                                w1q[:, f, k, :],
                                xt[:, k, :],
                                start=(k == 0),
                                stop=(k == KH - 1),
                            )
                        bias = (
                            b1t[:, fstart + f : fstart + f + 1] if use_b1 else 0.0
                        )
                        nc.scalar.activation(hq[:, f, :], pt1[:], gelu, bias=bias)

                    # GEMM2 (partial over this F-block):
                    # Y[t-tile, hh] += sum_k2 hT[k2, t-tile].T @ W2[k2, hh]
                    for t in range(nt // P):
                        pts = [
                            ps2.tile([P, 512], F32, tag="pt2", name=f"pt2_{hh}")
                            for hh in range(2)
                        ]
                        for k2 in range(fbn):
                            for hh in range(2):
                                nc.tensor.matmul(
                                    pts[hh][:],
                                    hq[:, k2, t * P : (t + 1) * P],
                                    w2q[:, k2, hh * 512 : (hh + 1) * 512],
                                    start=(k2 == 0),
                                    stop=(k2 == fbn - 1),
                                )
                        trow = coff // P + t
                        for hh in range(2):
                            ot = op.tile([P, 512], F32, tag="ot")
                            dst = yd[:, bi, trow, hh * 512 : (hh + 1) * 512]
                            nc.vector.tensor_copy(ot[:], pts[hh][:])
                            nc.sync.dma_start(dst, ot[:])
                    coff += nt
                fstart += fbn

    nc.compile()
    return nc


def _route(x2d, Wg):
    """Replicates reference router: softmax -> top-2 -> renormalize."""
    logits = x2d @ Wg  # [T, E] fp32
    m = logits.max(axis=-1, keepdims=True)
    p = np.exp(logits - m, dtype=np.float32)
    p /= p.sum(axis=-1, keepdims=True)
    # jax.lax.top_k: values descending, ties broken by lower index.
    order = np.argsort(-p, axis=-1, kind="stable")
    top_i = order[:, :TOP_K]  # [T, 2]
    top_p = np.take_along_axis(p, top_i, axis=-1)
    top_p = top_p / top_p.sum(axis=-1, keepdims=True)
    return top_i, top_p


def kernel(x, Wg, W1, b1, W2, b2):
    global LAST_RESULT
    x = np.ascontiguousarray(np.asarray(x, dtype=np.float32))
    Wg = np.ascontiguousarray(np.asarray(Wg, dtype=np.float32))
    W1 = np.ascontiguousarray(np.asarray(W1, dtype=np.float32))
    b1 = np.ascontiguousarray(np.asarray(b1, dtype=np.float32))
    W2 = np.ascontiguousarray(np.asarray(W2, dtype=np.float32))
    b2 = np.ascontiguousarray(np.asarray(b2, dtype=np.float32))

    x2d = x.reshape(T, H)
    top_i, top_p = _route(x2d, Wg)

    rows = [None] * E
    gval = [None] * E
    for e in range(E):
        r, slot = np.nonzero(top_i == e)
        rows[e] = r
        gval[e] = top_p[r, slot]

    c_max = max(len(r) for r in rows)
    c_pad = max(512, ((c_max + 255) // 256) * 256)
    chunks = _chunks_for(c_pad)
    use_b1 = bool(np.any(b1))

    mm_dt = {
        "fp32r": F32R,
        "fp32": F32,
        "bf16": mybir.dt.bfloat16,
    }[os.environ.get("KERNEL_MMDT", "fp32r")]
    key = (c_pad, chunks, use_b1, str(mm_dt))
    if key not in _CACHE:
        _CACHE[key] = _build(c_pad, chunks, use_b1, mm_dt)
    nc = _CACHE[key]

    np_dt = mybir.dt.np(mm_dt)
    in_maps = []
    for e in range(E):
        ce = len(rows[e])
        xt = np.zeros((H, c_pad), np.float32)
        xt[:, :ce] = x2d[rows[e]].T
        m = {
            "xd": np.ascontiguousarray(
                xt.reshape(KH, P, c_pad).transpose(1, 0, 2).astype(np_dt)
            ),
            "w1d": np.ascontiguousarray(
                W1[e].reshape(KH, P, FT, P).transpose(1, 2, 0, 3).astype(np_dt)
            ),
            "w2d": np.ascontiguousarray(
                W2[e].reshape(FT, P, H).transpose(1, 0, 2).astype(np_dt)
            ),
        }
        if use_b1:
            m["b1d"] = np.ascontiguousarray(b1[e].reshape(FT, P).T)
        in_maps.append(m)

    trace = os.environ.get("KERNEL_TRACE", "") == "1"
    res = run_bass_kernel_spmd(
        nc,
        in_maps,
        core_ids=list(range(N_CORES)),
        trace=trace,
        trace_cores=[0] if trace else None,
    )
    LAST_RESULT = res

    out = np.zeros((T, H), np.float32)
    for e in range(E):
        ce = len(rows[e])
        yt = res.results[e]["yd"].sum(axis=1, dtype=np.float32)  # [P, c_pad//P, H]
        y = yt.transpose(1, 0, 2).reshape(c_pad, H)[:ce]
        out[rows[e]] += gval[e][:, None] * (y + b2[e][None, :])

    return out.reshape(B, S, H)



# revision 2
# speedup vs baseline: 1.0408x; 1.0408x over previous
"""MoE FFN (top-2 of 8 experts) Trainium2 kernel.

Strategy (expert-parallel across 8 NeuronCores):
  - Host computes the (tiny) router: logits = x@Wg, softmax, top-2,
    renormalized combine weights.  Tokens are gathered per expert on the
    host ("all-to-all dispatch" done at sharding time), transposed to
    [H, C] so both FFN GEMMs run with natural weight layouts on device.
  - Everything on device is bf16 (same full-rate PE speed as fp32r,
    half the DMA bytes; PSUM accumulation stays fp32).  Both expert
    weight matrices fit in SBUF in bf16 (128 KB/partition), so they are
    loaded ONCE and stay resident:
      * x is read once (no per-F-block re-reads),
      * GEMM2 contracts over all of F in a single PSUM accumulation,
        so the output is written once (no per-block partials, no host
        summing).
  - Token chunks of 256; GEMM2 of chunk c is issued after GEMM1 of
    chunk c+1, so the PE fills the initial W2-streaming window with
    GEMM1 work and never stalls on the weight DMA head.
  - Host applies combine weights + b2 and scatter-adds back ("combine").

The kernel is compiled once per (c_pad, biases-zero, dtype)
configuration and cached in-process.
"""

import os
import sys
import numpy as np

for _p in ("/opt/trn_rl_repo", "/root/.axon_site/_ro/trn_rl_repo"):
    if _p not in sys.path and os.path.isdir(_p):
        sys.path.append(_p)

import concourse.bacc as bacc  # noqa: E402
import concourse.tile as tile  # noqa: E402
from concourse import mybir  # noqa: E402
from concourse.bass_utils import run_bass_kernel_spmd  # noqa: E402

# Problem shapes (hardcoded per spec)
B, S, H, F, E = 4, 2048, 1024, 4096, 8
T = B * S
TOP_K = 2
N_CORES = 8
P = 128
KH = H // P          # 8   H-contraction subtiles
FT = F // P          # 32  f-tiles total
CHUNK = 256          # tokens per GEMM1 chunk

F32 = mybir.dt.float32
BF16 = mybir.dt.bfloat16

_CACHE: dict = {}
LAST_RESULT = None  # BassKernelResults of the most recent run (for test.py)


def _build(c_pad: int, use_b1: bool, mm_dt):
    nc = bacc.Bacc(
        "TRN2",
        target_bir_lowering=False,
        debug=False,
        enable_asserts=False,
        num_devices=N_CORES,
    )

    n_chunks = c_pad // CHUNK

    xd = nc.dram_tensor("xd", [P, KH, c_pad], mm_dt, kind="ExternalInput").ap()
    w1d = nc.dram_tensor("w1d", [P, FT, KH, P], mm_dt, kind="ExternalInput").ap()
    w2d = nc.dram_tensor("w2d", [P, FT, H], mm_dt, kind="ExternalInput").ap()
    if use_b1:
        b1d = nc.dram_tensor("b1d", [P, FT], F32, kind="ExternalInput").ap()
    yd = nc.dram_tensor("yd", [P, c_pad // P, H], F32, kind="ExternalOutput").ap()

    gelu = mybir.ActivationFunctionType.Gelu_apprx_tanh

    with tile.TileContext(nc) as tc:
        with (
            tc.tile_pool(name="w1p", bufs=1) as w1p,
            tc.tile_pool(name="w2p", bufs=1) as w2p,
            tc.tile_pool(name="xp", bufs=3) as xp,
            tc.tile_pool(name="hp", bufs=3) as hp,
            tc.tile_pool(name="op", bufs=4) as op,
            tc.tile_pool(name="bp", bufs=1) as bp,
            tc.tile_pool(name="ps1", bufs=3, space="PSUM") as ps1,
            tc.tile_pool(name="ps2", bufs=4, space="PSUM") as ps2,
        ):
            if use_b1:
                b1t = bp.tile([P, FT], F32)
                nc.sync.dma_start(b1t[:], b1d[:])

            # resident weights, streamed in 1MB slices on the scalar ring
            w1 = w1p.tile([P, FT, KH, P], mm_dt)
            for i in range(0, FT, 4):
                nc.scalar.dma_start(w1[:, i : i + 4], w1d[:, i : i + 4])
            w2 = w2p.tile([P, FT, H], mm_dt)
            for i in range(0, FT, 4):
                nc.scalar.dma_start(w2[:, i : i + 4], w2d[:, i : i + 4])

            hqs = [None] * n_chunks

            def gemm1(ci):
                coff = ci * CHUNK
                xt = xp.tile([P, KH, CHUNK], mm_dt, tag="xt", name=f"xt_{ci}")
                nc.sync.dma_start(xt[:, :4], xd[:, :4, coff : coff + CHUNK])
                nc.sync.dma_start(xt[:, 4:], xd[:, 4:, coff : coff + CHUNK])
                hq = hp.tile([P, FT, CHUNK], mm_dt, tag="hq", name=f"hq_{ci}")
                hqs[ci] = hq
                for f in range(FT):
                    pt1 = ps1.tile([P, CHUNK], F32, tag="pt1")
                    for k in range(KH):
                        nc.tensor.matmul(
                            pt1[:],
                            w1[:, f, k, :],
                            xt[:, k, :],
                            start=(k == 0),
                            stop=(k == KH - 1),
                        )
                    bias = b1t[:, f : f + 1] if use_b1 else 0.0
                    nc.scalar.activation(hq[:, f, :], pt1[:], gelu, bias=bias)

            def gemm2(ci):
                hq = hqs[ci]
                for t in range(CHUNK // P):
                    trow = ci * (CHUNK // P) + t
                    pts = [
                        ps2.tile([P, 512], F32, tag="pt2", name=f"pt2_{hh}")
                        for hh in range(2)
                    ]
                    for k2 in range(FT):
                        for hh in range(2):
                            nc.tensor.matmul(
                                pts[hh][:],
                                hq[:, k2, t * P : (t + 1) * P],
                                w2[:, k2, hh * 512 : (hh + 1) * 512],
                                start=(k2 == 0),
                                stop=(k2 == FT - 1),
                            )
                    for hh in range(2):
                        ot = op.tile([P, 512], F32, tag="ot")
                        nc.vector.tensor_copy(ot[:], pts[hh][:])
                        nc.sync.dma_start(
                            yd[:, trow, hh * 512 : (hh + 1) * 512], ot[:]
                        )
                hqs[ci] = None

            # software pipeline: GEMM2 lags GEMM1 by one chunk so the PE
            # has GEMM1 work while the W2 stream finishes.
            for ci in range(n_chunks):
                gemm1(ci)
                if ci >= 1:
                    gemm2(ci - 1)
            gemm2(n_chunks - 1)

    nc.compile()
    return nc


def _route(x2d, Wg):
    """Replicates reference router: softmax -> top-2 -> renormalize."""
    logits = x2d @ Wg  # [T, E] fp32
    m = logits.max(axis=-1, keepdims=True)
    p = np.exp(logits - m, dtype=np.float32)
    p /= p.sum(axis=-1, keepdims=True)
    # jax.lax.top_k: values descending, ties broken by lower index.
    order = np.argsort(-p, axis=-1, kind="stable")
    top_i = order[:, :TOP_K]  # [T, 2]
    top_p = np.take_along_axis(p, top_i, axis=-1)
    top_p = top_p / top_p.sum(axis=-1, keepdims=True)
    return top_i, top_p


def kernel(x, Wg, W1, b1, W2, b2):
    global LAST_RESULT
    x = np.ascontiguousarray(np.asarray(x, dtype=np.float32))
    Wg = np.ascontiguousarray(np.asarray(Wg, dtype=np.float32))
    W1 = np.ascontiguousarray(np.asarray(W1, dtype=np.float32))
    b1 = np.ascontiguousarray(np.asarray(b1, dtype=np.float32))
    W2 = np.ascontiguousarray(np.asarray(W2, dtype=np.float32))
    b2 = np.ascontiguousarray(np.asarray(b2, dtype=np.float32))

    x2d = x.reshape(T, H)
    top_i, top_p = _route(x2d, Wg)

    rows = [None] * E
    gval = [None] * E
    for e in range(E):
        r, slot = np.nonzero(top_i == e)
        rows[e] = r
        gval[e] = top_p[r, slot]

    c_max = max(len(r) for r in rows)
    c_pad = max(CHUNK, ((c_max + CHUNK - 1) // CHUNK) * CHUNK)
    use_b1 = bool(np.any(b1))

    mm_dt = {
        "bf16": BF16,
        "fp32": F32,
    }[os.environ.get("KERNEL_MMDT", "bf16")]
    key = (c_pad, use_b1, str(mm_dt))
    if key not in _CACHE:
        _CACHE[key] = _build(c_pad, use_b1, mm_dt)
    nc = _CACHE[key]

    np_dt = mybir.dt.np(mm_dt)
    in_maps = []
    for e in range(E):
        ce = len(rows[e])
        xt = np.zeros((H, c_pad), np.float32)
        xt[:, :ce] = x2d[rows[e]].T
        m = {
            "xd": np.ascontiguousarray(
                xt.reshape(KH, P, c_pad).transpose(1, 0, 2).astype(np_dt)
            ),
            "w1d": np.ascontiguousarray(
                W1[e].reshape(KH, P, FT, P).transpose(1, 2, 0, 3).astype(np_dt)
            ),
            "w2d": np.ascontiguousarray(
                W2[e].reshape(FT, P, H).transpose(1, 0, 2).astype(np_dt)
            ),
        }
        if use_b1:
            m["b1d"] = np.ascontiguousarray(b1[e].reshape(FT, P).T)
        in_maps.append(m)

    trace = os.environ.get("KERNEL_TRACE", "") == "1"
    res = run_bass_kernel_spmd(
        nc,
        in_maps,
        core_ids=list(range(N_CORES)),
        trace=trace,
        trace_cores=[0] if trace else None,
    )
    LAST_RESULT = res

    out = np.zeros((T, H), np.float32)
    for e in range(E):
        ce = len(rows[e])
        yt = res.results[e]["yd"]  # [P, c_pad//P, H]
        y = yt.transpose(1, 0, 2).reshape(c_pad, H)[:ce]
        out[rows[e]] += gval[e][:, None] * (y + b2[e][None, :])

    return out.reshape(B, S, H)


# revision 5
# speedup vs baseline: 1.0500x; 1.0088x over previous
"""MoE FFN (top-2 of 8 experts) Trainium2 kernel.

Strategy (expert-parallel across 8 NeuronCores):
  - Host computes the (tiny) router: logits = x@Wg, softmax, top-2,
    renormalized combine weights.  Tokens are gathered per expert on the
    host ("all-to-all dispatch" done at sharding time), transposed to
    [H, C] so both FFN GEMMs run with natural weight layouts on device.
  - Everything on device is bf16 (same full-rate PE speed as fp32r,
    half the DMA bytes; PSUM accumulation stays fp32).  Both expert
    weight matrices fit in SBUF in bf16 (128 KB/partition), so they are
    loaded ONCE and stay resident:
      * x is read once (no per-F-block re-reads),
      * GEMM2 contracts over all of F in a single PSUM accumulation,
        so the output is written once (no per-block partials, no host
        summing).
  - Token chunks of 256; GEMM2 of chunk c is issued after GEMM1 of
    chunk c+1, so the PE fills the initial W2-streaming window with
    GEMM1 work and never stalls on the weight DMA head.
  - Host applies combine weights + b2 and scatter-adds back ("combine").

The kernel is compiled once per (c_pad, biases-zero, dtype)
configuration and cached in-process.
"""

import os
import sys
import numpy as np

for _p in ("/opt/trn_rl_repo", "/root/.axon_site/_ro/trn_rl_repo"):
    if _p not in sys.path and os.path.isdir(_p):
        sys.path.append(_p)

import concourse.bacc as bacc  # noqa: E402
import concourse.tile as tile  # noqa: E402
from concourse import mybir  # noqa: E402
from concourse.bass_utils import run_bass_kernel_spmd  # noqa: E402

# Problem shapes (hardcoded per spec)
B, S, H, F, E = 4, 2048, 1024, 4096, 8
T = B * S
TOP_K = 2
N_CORES = 8
P = 128
KH = H // P          # 8   H-contraction subtiles
FT = F // P          # 32  f-tiles total
CHUNK = 256          # tokens per GEMM1 chunk

F32 = mybir.dt.float32
BF16 = mybir.dt.bfloat16

_CACHE: dict = {}
LAST_RESULT = None  # BassKernelResults of the most recent run (for test.py)


def _build(c_pad: int, use_b1: bool, mm_dt):
    nc = bacc.Bacc(
        "TRN2",
        target_bir_lowering=False,
        debug=False,
        enable_asserts=False,
        num_devices=N_CORES,
    )

    n_chunks = c_pad // CHUNK

    # chunk-major x layout: one chunk = 4KB contiguous per partition, so a
    # chunk DMA is 128 large descriptors instead of 1024 strided 512B ones
    xd = nc.dram_tensor(
        "xd", [P, n_chunks, KH, CHUNK], mm_dt, kind="ExternalInput"
    ).ap()
    w1d = nc.dram_tensor("w1d", [P, FT, KH, P], mm_dt, kind="ExternalInput").ap()
    w2d = nc.dram_tensor("w2d", [P, FT, H], mm_dt, kind="ExternalInput").ap()
    if use_b1:
        b1d = nc.dram_tensor("b1d", [P, FT], F32, kind="ExternalInput").ap()
    yd = nc.dram_tensor("yd", [P, c_pad // P, H], F32, kind="ExternalOutput").ap()

    gelu = mybir.ActivationFunctionType.Gelu_apprx_tanh

    with tile.TileContext(nc) as tc:
        with (
            tc.tile_pool(name="w1p", bufs=1) as w1p,
            tc.tile_pool(name="w2p", bufs=1) as w2p,
            tc.tile_pool(name="xp", bufs=3) as xp,
            tc.tile_pool(name="hp", bufs=3) as hp,
            tc.tile_pool(name="op", bufs=4) as op,
            tc.tile_pool(name="bp", bufs=1) as bp,
            tc.tile_pool(name="ps1", bufs=3, space="PSUM") as ps1,
            tc.tile_pool(name="ps2", bufs=4, space="PSUM") as ps2,
        ):
            if use_b1:
                b1t = bp.tile([P, FT], F32)
                nc.sync.dma_start(b1t[:], b1d[:])

            # resident weights, streamed in 1MB slices on the scalar ring
            w1 = w1p.tile([P, FT, KH, P], mm_dt)
            for i in range(0, FT, 4):
                nc.scalar.dma_start(w1[:, i : i + 4], w1d[:, i : i + 4])
            w2 = w2p.tile([P, FT, H], mm_dt)
            for i in range(0, FT, 4):
                nc.scalar.dma_start(w2[:, i : i + 4], w2d[:, i : i + 4])

            hqs = [None] * n_chunks

            def gemm1(ci):
                xt = xp.tile([P, KH, CHUNK], mm_dt, tag="xt", name=f"xt_{ci}")
                nc.sync.dma_start(xt[:, :4], xd[:, ci, :4])
                nc.sync.dma_start(xt[:, 4:], xd[:, ci, 4:])
                hq = hp.tile([P, FT, CHUNK], mm_dt, tag="hq", name=f"hq_{ci}")
                hqs[ci] = hq
                for f in range(FT):
                    pt1 = ps1.tile([P, CHUNK], F32, tag="pt1")
                    for k in range(KH):
                        nc.tensor.matmul(
                            pt1[:],
                            w1[:, f, k, :],
                            xt[:, k, :],
                            start=(k == 0),
                            stop=(k == KH - 1),
                        )
                    bias = b1t[:, f : f + 1] if use_b1 else 0.0
                    nc.scalar.activation(hq[:, f, :], pt1[:], gelu, bias=bias)

            def gemm2(ci):
                hq = hqs[ci]
                for t in range(CHUNK // P):
                    trow = ci * (CHUNK // P) + t
                    pts = [
                        ps2.tile([P, 512], F32, tag="pt2", name=f"pt2_{hh}")
                        for hh in range(2)
                    ]
                    for k2 in range(FT):
                        for hh in range(2):
                            nc.tensor.matmul(
                                pts[hh][:],
                                hq[:, k2, t * P : (t + 1) * P],
                                w2[:, k2, hh * 512 : (hh + 1) * 512],
                                start=(k2 == 0),
                                stop=(k2 == FT - 1),
                            )
                    for hh in range(2):
                        ot = op.tile([P, 512], F32, tag="ot")
                        nc.vector.tensor_copy(ot[:], pts[hh][:])
                        nc.sync.dma_start(
                            yd[:, trow, hh * 512 : (hh + 1) * 512], ot[:]
                        )
                hqs[ci] = None

            # software pipeline: GEMM2 lags GEMM1 by one chunk so the PE
            # has GEMM1 work while the W2 stream finishes.
            for ci in range(n_chunks):
                gemm1(ci)
                if ci >= 1:
                    gemm2(ci - 1)
            gemm2(n_chunks - 1)

    nc.compile()
    return nc


def _route(x2d, Wg):
    """Replicates reference router: softmax -> top-2 -> renormalize."""
    logits = x2d @ Wg  # [T, E] fp32
    m = logits.max(axis=-1, keepdims=True)
    p = np.exp(logits - m, dtype=np.float32)
    p /= p.sum(axis=-1, keepdims=True)
    # jax.lax.top_k: values descending, ties broken by lower index.
    order = np.argsort(-p, axis=-1, kind="stable")
    top_i = order[:, :TOP_K]  # [T, 2]
    top_p = np.take_along_axis(p, top_i, axis=-1)
    top_p = top_p / top_p.sum(axis=-1, keepdims=True)
    return top_i, top_p


def kernel(x, Wg, W1, b1, W2, b2):
    global LAST_RESULT
    x = np.ascontiguousarray(np.asarray(x, dtype=np.float32))
    Wg = np.ascontiguousarray(np.asarray(Wg, dtype=np.float32))
    W1 = np.ascontiguousarray(np.asarray(W1, dtype=np.float32))
    b1 = np.ascontiguousarray(np.asarray(b1, dtype=np.float32))
    W2 = np.ascontiguousarray(np.asarray(W2, dtype=np.float32))
    b2 = np.ascontiguousarray(np.asarray(b2, dtype=np.float32))

    x2d = x.reshape(T, H)
    top_i, top_p = _route(x2d, Wg)

    rows = [None] * E
    gval = [None] * E
    for e in range(E):
        r, slot = np.nonzero(top_i == e)
        rows[e] = r
        gval[e] = top_p[r, slot]

    c_max = max(len(r) for r in rows)
    c_pad = max(CHUNK, ((c_max + CHUNK - 1) // CHUNK) * CHUNK)
    use_b1 = bool(np.any(b1))

    mm_dt = {
        "bf16": BF16,
        "fp32": F32,
    }[os.environ.get("KERNEL_MMDT", "bf16")]
    key = (c_pad, use_b1, str(mm_dt))
    if key not in _CACHE:
        _CACHE[key] = _build(c_pad, use_b1, mm_dt)
    nc = _CACHE[key]

    np_dt = mybir.dt.np(mm_dt)
    in_maps = []
    for e in range(E):
        ce = len(rows[e])
        xt = np.zeros((H, c_pad), np.float32)
        xt[:, :ce] = x2d[rows[e]].T
        n_chunks = c_pad // CHUNK
        m = {
            "xd": np.ascontiguousarray(
                xt.reshape(KH, P, n_chunks, CHUNK)
                .transpose(1, 2, 0, 3)
                .astype(np_dt)
            ),
            "w1d": np.ascontiguousarray(
                W1[e].reshape(KH, P, FT, P).transpose(1, 2, 0, 3).astype(np_dt)
            ),
            "w2d": np.ascontiguousarray(
                W2[e].reshape(FT, P, H).transpose(1, 0, 2).astype(np_dt)
            ),
        }
        if use_b1:
            m["b1d"] = np.ascontiguousarray(b1[e].reshape(FT, P).T)
        in_maps.append(m)

    trace = os.environ.get("KERNEL_TRACE", "") == "1"
    res = run_bass_kernel_spmd(
        nc,
        in_maps,
        core_ids=list(range(N_CORES)),
        trace=trace,
        trace_cores=[0] if trace else None,
    )
    LAST_RESULT = res

    out = np.zeros((T, H), np.float32)
    for e in range(E):
        ce = len(rows[e])
        yt = res.results[e]["yd"]  # [P, c_pad//P, H]
        y = yt.transpose(1, 0, 2).reshape(c_pad, H)[:ce]
        out[rows[e]] += gval[e][:, None] * (y + b2[e][None, :])

    return out.reshape(B, S, H)


# revision 7
# speedup vs baseline: 1.0856x; 1.0339x over previous
"""MoE FFN (top-2 of 8 experts) Trainium2 kernel.

Strategy (expert-parallel across 8 NeuronCores):
  - Host computes the (tiny) router: logits = x@Wg, softmax, top-2,
    renormalized combine weights.  Tokens are gathered per expert on the
    host ("all-to-all dispatch" done at sharding time), transposed to
    [H, C] so both FFN GEMMs run with natural weight layouts on device.
  - Everything on device is bf16 (same full-rate PE speed as fp32r,
    half the DMA bytes; PSUM accumulation stays fp32).  Both expert
    weight matrices fit in SBUF in bf16 (128 KB/partition), so they are
    loaded ONCE and stay resident:
      * x is read once (no per-F-block re-reads),
      * GEMM2 contracts over all of F in a single PSUM accumulation,
        so the output is written once (no per-block partials, no host
        summing).
  - Token chunks of 256; GEMM2 of chunk c is issued after GEMM1 of
    chunk c+1, so the PE fills the initial W2-streaming window with
    GEMM1 work and never stalls on the weight DMA head.
  - Host applies combine weights + b2 and scatter-adds back ("combine").

The kernel is compiled once per (c_pad, biases-zero, dtype)
configuration and cached in-process.
"""

import os
import sys
import numpy as np

for _p in ("/opt/trn_rl_repo", "/root/.axon_site/_ro/trn_rl_repo"):
    if _p not in sys.path and os.path.isdir(_p):
        sys.path.append(_p)

import concourse.bacc as bacc  # noqa: E402
import concourse.tile as tile  # noqa: E402
from concourse import mybir  # noqa: E402
from concourse.bass_utils import run_bass_kernel_spmd  # noqa: E402

# Problem shapes (hardcoded per spec)
B, S, H, F, E = 4, 2048, 1024, 4096, 8
T = B * S
TOP_K = 2
N_CORES = 8
P = 128
KH = H // P          # 8   H-contraction subtiles
FT = F // P          # 32  f-tiles total
CHUNK = 256          # tokens per GEMM1 chunk

F32 = mybir.dt.float32
BF16 = mybir.dt.bfloat16

_CACHE: dict = {}
LAST_RESULT = None  # BassKernelResults of the most recent run (for test.py)


def _build(c_pad: int, use_b1: bool, mm_dt):
    nc = bacc.Bacc(
        "TRN2",
        target_bir_lowering=False,
        debug=False,
        enable_asserts=False,
        num_devices=N_CORES,
    )

    n_chunks = c_pad // CHUNK

    # chunk-major x layout: one chunk = 4KB contiguous per partition, so a
    # chunk DMA is 128 large descriptors instead of 1024 strided 512B ones
    xd = nc.dram_tensor(
        "xd", [P, n_chunks, KH, CHUNK], mm_dt, kind="ExternalInput"
    ).ap()
    w1d = nc.dram_tensor("w1d", [P, FT, KH, P], mm_dt, kind="ExternalInput").ap()
    w2d = nc.dram_tensor("w2d", [P, FT, H], mm_dt, kind="ExternalInput").ap()
    if use_b1:
        b1d = nc.dram_tensor("b1d", [P, FT], F32, kind="ExternalInput").ap()
    yd = nc.dram_tensor("yd", [P, c_pad // P, H], F32, kind="ExternalOutput").ap()

    gelu = mybir.ActivationFunctionType.Gelu_apprx_tanh

    with tile.TileContext(nc) as tc:
        with (
            tc.tile_pool(name="w1p", bufs=1) as w1p,
            tc.tile_pool(name="w2p", bufs=1) as w2p,
            tc.tile_pool(name="xp", bufs=3) as xp,
            tc.tile_pool(name="hp", bufs=3) as hp,
            tc.tile_pool(name="op", bufs=4) as op,
            tc.tile_pool(name="bp", bufs=1) as bp,
            tc.tile_pool(name="ps1", bufs=3, space="PSUM") as ps1,
            tc.tile_pool(name="ps2", bufs=4, space="PSUM") as ps2,
        ):
            if use_b1:
                b1t = bp.tile([P, FT], F32)
                nc.sync.dma_start(b1t[:], b1d[:])

            # resident weights on the scalar ring; slice sizes graded so the
            # first GEMM1 f-tiles can start ~5us in while later slices land
            # at the ~0.86us/f-tile consumption pace
            W1_SLICES = (2, 2, 4, 8, 8, 8)
            w1 = w1p.tile([P, FT, KH, P], mm_dt)
            i = 0
            for n in W1_SLICES:
                nc.scalar.dma_start(w1[:, i : i + n], w1d[:, i : i + n])
                i += n
            w2 = w2p.tile([P, FT, H], mm_dt)
            for i in range(0, FT, 8):
                nc.scalar.dma_start(w2[:, i : i + 8], w2d[:, i : i + 8])

            hqs = [None] * n_chunks

            def gemm1(ci):
                xt = xp.tile([P, KH, CHUNK], mm_dt, tag="xt", name=f"xt_{ci}")
                nc.sync.dma_start(xt[:], xd[:, ci])
                hq = hp.tile([P, FT, CHUNK], mm_dt, tag="hq", name=f"hq_{ci}")
                hqs[ci] = hq
                for f in range(FT):
                    pt1 = ps1.tile([P, CHUNK], F32, tag="pt1")
                    for k in range(KH):
                        nc.tensor.matmul(
                            pt1[:],
                            w1[:, f, k, :],
                            xt[:, k, :],
                            start=(k == 0),
                            stop=(k == KH - 1),
                        )
                    bias = b1t[:, f : f + 1] if use_b1 else 0.0
                    nc.scalar.activation(hq[:, f, :], pt1[:], gelu, bias=bias)

            def gemm2(ci):
                hq = hqs[ci]
                for t in range(CHUNK // P):
                    trow = ci * (CHUNK // P) + t
                    pts = [
                        ps2.tile([P, 512], F32, tag="pt2", name=f"pt2_{hh}")
                        for hh in range(2)
                    ]
                    for k2 in range(FT):
                        for hh in range(2):
                            nc.tensor.matmul(
                                pts[hh][:],
                                hq[:, k2, t * P : (t + 1) * P],
                                w2[:, k2, hh * 512 : (hh + 1) * 512],
                                start=(k2 == 0),
                                stop=(k2 == FT - 1),
                            )
                    for hh in range(2):
                        ot = op.tile([P, 512], F32, tag="ot")
                        nc.vector.tensor_copy(ot[:], pts[hh][:])
                        nc.sync.dma_start(
                            yd[:, trow, hh * 512 : (hh + 1) * 512], ot[:]
                        )
                hqs[ci] = None

            # software pipeline: GEMM2 lags GEMM1 by one chunk so the PE
            # has GEMM1 work while the W2 stream finishes.
            for ci in range(n_chunks):
                gemm1(ci)
                if ci >= 1:
                    gemm2(ci - 1)
            gemm2(n_chunks - 1)

    nc.compile()
    return nc


def _route(x2d, Wg):
    """Replicates reference router: softmax -> top-2 -> renormalize."""
    logits = x2d @ Wg  # [T, E] fp32
    m = logits.max(axis=-1, keepdims=True)
    p = np.exp(logits - m, dtype=np.float32)
    p /= p.sum(axis=-1, keepdims=True)
    # jax.lax.top_k: values descending, ties broken by lower index.
    order = np.argsort(-p, axis=-1, kind="stable")
    top_i = order[:, :TOP_K]  # [T, 2]
    top_p = np.take_along_axis(p, top_i, axis=-1)
    top_p = top_p / top_p.sum(axis=-1, keepdims=True)
    return top_i, top_p


def kernel(x, Wg, W1, b1, W2, b2):
    global LAST_RESULT
    x = np.ascontiguousarray(np.asarray(x, dtype=np.float32))
    Wg = np.ascontiguousarray(np.asarray(Wg, dtype=np.float32))
    W1 = np.ascontiguousarray(np.asarray(W1, dtype=np.float32))
    b1 = np.ascontiguousarray(np.asarray(b1, dtype=np.float32))
    W2 = np.ascontiguousarray(np.asarray(W2, dtype=np.float32))
    b2 = np.ascontiguousarray(np.asarray(b2, dtype=np.float32))

    x2d = x.reshape(T, H)
    top_i, top_p = _route(x2d, Wg)

    rows = [None] * E
    gval = [None] * E
    for e in range(E):
        r, slot = np.nonzero(top_i == e)
        rows[e] = r
        gval[e] = top_p[r, slot]

    c_max = max(len(r) for r in rows)
    c_pad = max(CHUNK, ((c_max + CHUNK - 1) // CHUNK) * CHUNK)
    use_b1 = bool(np.any(b1))

    mm_dt = {
        "bf16": BF16,
        "fp32": F32,
    }[os.environ.get("KERNEL_MMDT", "bf16")]
    key = (c_pad, use_b1, str(mm_dt))
    if key not in _CACHE:
        _CACHE[key] = _build(c_pad, use_b1, mm_dt)
    nc = _CACHE[key]

    np_dt = mybir.dt.np(mm_dt)
    in_maps = []
    for e in range(E):
        ce = len(rows[e])
        xt = np.zeros((H, c_pad), np.float32)
        xt[:, :ce] = x2d[rows[e]].T
        n_chunks = c_pad // CHUNK
        m = {
            "xd": np.ascontiguousarray(
                xt.reshape(KH, P, n_chunks, CHUNK)
                .transpose(1, 2, 0, 3)
                .astype(np_dt)
            ),
            "w1d": np.ascontiguousarray(
                W1[e].reshape(KH, P, FT, P).transpose(1, 2, 0, 3).astype(np_dt)
            ),
            "w2d": np.ascontiguousarray(
                W2[e].reshape(FT, P, H).transpose(1, 0, 2).astype(np_dt)
            ),
        }
        if use_b1:
            m["b1d"] = np.ascontiguousarray(b1[e].reshape(FT, P).T)
        in_maps.append(m)

    trace = os.environ.get("KERNEL_TRACE", "") == "1"
    res = run_bass_kernel_spmd(
        nc,
        in_maps,
        core_ids=list(range(N_CORES)),
        trace=trace,
        trace_cores=[0] if trace else None,
    )
    LAST_RESULT = res

    out = np.zeros((T, H), np.float32)
    for e in range(E):
        ce = len(rows[e])
        yt = res.results[e]["yd"]  # [P, c_pad//P, H]
        y = yt.transpose(1, 0, 2).reshape(c_pad, H)[:ce]
        out[rows[e]] += gval[e][:, None] * (y + b2[e][None, :])

    return out.reshape(B, S, H)


# revision 8
# speedup vs baseline: 1.1482x; 1.0577x over previous
"""MoE FFN (top-2 of 8 experts) Trainium2 kernel, pair/F-split variant.

Same host-side routing as the resident-weight kernel, but load-balanced:
experts are sorted by token count and paired heavy-with-light; pair p is
assigned to cores (2p, 2p+1).  Each core of the pair holds HALF the F
dimension of BOTH experts' weights (same 16.8 MB bf16 SBUF footprint as
one full expert) and processes ALL of the pair's tokens at half F:

    per-core slots = ca_pad + cb_pad  (~4352 half-F token slots)
                   = ~2176 full-token equivalents vs 2304 for pure
                     expert-parallel -> ~6% less PE work.

GEMM2 contracts only the local F half, so each token's output is a
partial; the host sums the two cores' partials ("combine" still on
host).  Everything else (bf16, resident weights, chunk-major x, lag-1
GEMM2 pipeline, graded weight-slice streaming) matches the resident
kernel.
"""

import os
import sys
import numpy as np

for _p in ("/opt/trn_rl_repo", "/root/.axon_site/_ro/trn_rl_repo"):
    if _p not in sys.path and os.path.isdir(_p):
        sys.path.append(_p)

import concourse.bacc as bacc  # noqa: E402
import concourse.tile as tile  # noqa: E402
from concourse import mybir  # noqa: E402
from concourse.bass_utils import run_bass_kernel_spmd  # noqa: E402

# Problem shapes (hardcoded per spec)
B, S, H, F, E = 4, 2048, 1024, 4096, 8
T = B * S
TOP_K = 2
N_CORES = 8
P = 128
KH = H // P          # 8   H-contraction subtiles
F2 = F // 2          # 2048 F per core (half of one expert)
FT2 = F2 // P        # 16  f-tiles per expert half
CHUNK = 256          # tokens per GEMM1 chunk

F32 = mybir.dt.float32
BF16 = mybir.dt.bfloat16

_CACHE: dict = {}
LAST_RESULT = None  # BassKernelResults of the most recent run (for test.py)


def _build(ca_pad: int, cb_pad: int, use_b1: bool, mm_dt):
    nc = bacc.Bacc(
        "TRN2",
        target_bir_lowering=False,
        debug=False,
        enable_asserts=False,
        num_devices=N_CORES,
    )

    na = ca_pad // CHUNK
    nb = cb_pad // CHUNK
    n_chunks = na + nb
    c_pad = ca_pad + cb_pad
    NF = 2 * FT2  # 32 f-tiles total (16 per expert)

    xd = nc.dram_tensor(
        "xd", [P, n_chunks, KH, CHUNK], mm_dt, kind="ExternalInput"
    ).ap()
    w1d = nc.dram_tensor("w1d", [P, NF, KH, P], mm_dt, kind="ExternalInput").ap()
    w2d = nc.dram_tensor("w2d", [P, NF, H], mm_dt, kind="ExternalInput").ap()
    if use_b1:
        b1d = nc.dram_tensor("b1d", [P, NF], F32, kind="ExternalInput").ap()
    yd = nc.dram_tensor("yd", [P, c_pad // P, H], F32, kind="ExternalOutput").ap()

    gelu = mybir.ActivationFunctionType.Gelu_apprx_tanh

    with tile.TileContext(nc) as tc:
        with (
            tc.tile_pool(name="w1p", bufs=1) as w1p,
            tc.tile_pool(name="w2p", bufs=1) as w2p,
            tc.tile_pool(name="xp", bufs=3) as xp,
            tc.tile_pool(name="hp", bufs=3) as hp,
            tc.tile_pool(name="op", bufs=4) as op,
            tc.tile_pool(name="bp", bufs=1) as bp,
            tc.tile_pool(name="ps1", bufs=3, space="PSUM") as ps1,
            tc.tile_pool(name="ps2", bufs=4, space="PSUM") as ps2,
        ):
            if use_b1:
                b1t = bp.tile([P, NF], F32)
                nc.sync.dma_start(b1t[:], b1d[:])

            # weight stream order: w1(expert a) graded, w2(a), w1(b), w2(b)
            w1 = w1p.tile([P, NF, KH, P], mm_dt)
            w2 = w2p.tile([P, NF, H], mm_dt)

            def stream_w(eh):  # eh = 0 (expert a) or 1 (expert b)
                base = eh * FT2
                slices = (2, 2, 4, 8) if eh == 0 else (8, 8)
                i = base
                for n in slices:
                    nc.scalar.dma_start(w1[:, i : i + n], w1d[:, i : i + n])
                    i += n
                for i in range(base, base + FT2, 8):
                    nc.scalar.dma_start(w2[:, i : i + 8], w2d[:, i : i + 8])

            stream_w(0)
            stream_w(1)

            hqs = [None] * n_chunks

            def gemm1(ci):
                f0 = 0 if ci < na else FT2
                xt = xp.tile([P, KH, CHUNK], mm_dt, tag="xt", name=f"xt_{ci}")
                nc.sync.dma_start(xt[:], xd[:, ci])
                hq = hp.tile([P, FT2, CHUNK], mm_dt, tag="hq", name=f"hq_{ci}")
                hqs[ci] = hq
                for fi in range(FT2):
                    f = f0 + fi
                    pt1 = ps1.tile([P, CHUNK], F32, tag="pt1")
                    for k in range(KH):
                        nc.tensor.matmul(
                            pt1[:],
                            w1[:, f, k, :],
                            xt[:, k, :],
                            start=(k == 0),
                            stop=(k == KH - 1),
                        )
                    bias = b1t[:, f : f + 1] if use_b1 else 0.0
                    nc.scalar.activation(hq[:, fi, :], pt1[:], gelu, bias=bias)

            def gemm2(ci):
                f0 = 0 if ci < na else FT2
                hq = hqs[ci]
                for t in range(CHUNK // P):
                    trow = ci * (CHUNK // P) + t
                    pts = [
                        ps2.tile([P, 512], F32, tag="pt2", name=f"pt2_{hh}")
                        for hh in range(2)
                    ]
                    for k2 in range(FT2):
                        for hh in range(2):
                            nc.tensor.matmul(
                                pts[hh][:],
                                hq[:, k2, t * P : (t + 1) * P],
                                w2[:, f0 + k2, hh * 512 : (hh + 1) * 512],
                                start=(k2 == 0),
                                stop=(k2 == FT2 - 1),
                            )
                    for hh in range(2):
                        ot = op.tile([P, 512], F32, tag="ot")
                        nc.vector.tensor_copy(ot[:], pts[hh][:])
                        nc.sync.dma_start(
                            yd[:, trow, hh * 512 : (hh + 1) * 512], ot[:]
                        )
                hqs[ci] = None

            for ci in range(n_chunks):
                gemm1(ci)
                if ci >= 1:
                    gemm2(ci - 1)
            gemm2(n_chunks - 1)

    nc.compile()
    return nc


def _route(x2d, Wg):
    """Replicates reference router: softmax -> top-2 -> renormalize."""
    logits = x2d @ Wg  # [T, E] fp32
    m = logits.max(axis=-1, keepdims=True)
    p = np.exp(logits - m, dtype=np.float32)
    p /= p.sum(axis=-1, keepdims=True)
    # jax.lax.top_k: values descending, ties broken by lower index.
    order = np.argsort(-p, axis=-1, kind="stable")
    top_i = order[:, :TOP_K]  # [T, 2]
    top_p = np.take_along_axis(p, top_i, axis=-1)
    top_p = top_p / top_p.sum(axis=-1, keepdims=True)
    return top_i, top_p


def _pad_chunks(c: int) -> int:
    return max(CHUNK, ((c + CHUNK - 1) // CHUNK) * CHUNK)


def kernel(x, Wg, W1, b1, W2, b2):
    global LAST_RESULT
    x = np.ascontiguousarray(np.asarray(x, dtype=np.float32))
    Wg = np.ascontiguousarray(np.asarray(Wg, dtype=np.float32))
    W1 = np.ascontiguousarray(np.asarray(W1, dtype=np.float32))
    b1 = np.ascontiguousarray(np.asarray(b1, dtype=np.float32))
    W2 = np.ascontiguousarray(np.asarray(W2, dtype=np.float32))
    b2 = np.ascontiguousarray(np.asarray(b2, dtype=np.float32))

    x2d = x.reshape(T, H)
    top_i, top_p = _route(x2d, Wg)

    rows = [None] * E
    gval = [None] * E
    for e in range(E):
        r, slot = np.nonzero(top_i == e)
        rows[e] = r
        gval[e] = top_p[r, slot]

    counts = np.array([len(r) for r in rows])
    order = np.argsort(-counts, kind="stable")
    pairs = [(int(order[i]), int(order[E - 1 - i])) for i in range(E // 2)]
    ca_pad = _pad_chunks(int(counts[[p[0] for p in pairs]].max()))
    cb_pad = _pad_chunks(int(counts[[p[1] for p in pairs]].max()))
    use_b1 = bool(np.any(b1))

    mm_dt = {
        "bf16": BF16,
        "fp32": F32,
    }[os.environ.get("KERNEL_MMDT", "bf16")]
    key = (ca_pad, cb_pad, use_b1, str(mm_dt))
    if key not in _CACHE:
        _CACHE[key] = _build(ca_pad, cb_pad, use_b1, mm_dt)
    nc = _CACHE[key]

    np_dt = mybir.dt.np(mm_dt)
    c_pad = ca_pad + cb_pad
    n_chunks = c_pad // CHUNK

    def pack_w(e, h):
        lo, hi = h * F2, (h + 1) * F2
        w1p = np.ascontiguousarray(
            W1[e][:, lo:hi].reshape(KH, P, FT2, P).transpose(1, 2, 0, 3)
        )
        w2p = np.ascontiguousarray(
            W2[e][lo:hi, :].reshape(FT2, P, H).transpose(1, 0, 2)
        )
        return w1p.astype(np_dt), w2p.astype(np_dt)

    in_maps = [None] * N_CORES
    for pi, (a, b) in enumerate(pairs):
        xt = np.zeros((H, c_pad), np.float32)
        xt[:, : counts[a]] = x2d[rows[a]].T
        xt[:, ca_pad : ca_pad + counts[b]] = x2d[rows[b]].T
        xd = np.ascontiguousarray(
            xt.reshape(KH, P, n_chunks, CHUNK).transpose(1, 2, 0, 3).astype(np_dt)
        )
        for h in range(2):
            w1a, w2a = pack_w(a, h)
            w1b, w2b = pack_w(b, h)
            m = {
                "xd": xd,
                "w1d": np.ascontiguousarray(np.concatenate([w1a, w1b], axis=1)),
                "w2d": np.ascontiguousarray(np.concatenate([w2a, w2b], axis=1)),
            }
            if use_b1:
                lo, hi = h * F2, (h + 1) * F2
                m["b1d"] = np.ascontiguousarray(
                    np.concatenate(
                        [
                            b1[a][lo:hi].reshape(FT2, P).T,
                            b1[b][lo:hi].reshape(FT2, P).T,
                        ],
                        axis=1,
                    )
                )
            in_maps[2 * pi + h] = m

    trace = os.environ.get("KERNEL_TRACE", "") == "1"
    res = run_bass_kernel_spmd(
        nc,
        in_maps,
        core_ids=list(range(N_CORES)),
        trace=trace,
        trace_cores=[0] if trace else None,
    )
    LAST_RESULT = res

    out = np.zeros((T, H), np.float32)
    for pi, (a, b) in enumerate(pairs):
        y = res.results[2 * pi]["yd"] + res.results[2 * pi + 1]["yd"]
        y = y.transpose(1, 0, 2).reshape(c_pad, H)
        out[rows[a]] += gval[a][:, None] * (y[: counts[a]] + b2[a][None, :])
        out[rows[b]] += gval[b][:, None] * (
            y[ca_pad : ca_pad + counts[b]] + b2[b][None, :]
        )

    return out.reshape(B, S, H)


# revision 10
# speedup vs baseline: 1.1577x; 1.0083x over previous
"""MoE FFN (top-2 of 8 experts) Trainium2 kernel, pair/F-split variant.

Same host-side routing as the resident-weight kernel, but load-balanced:
experts are sorted by token count and paired heavy-with-light; pair p is
assigned to cores (2p, 2p+1).  Each core of the pair holds HALF the F
dimension of BOTH experts' weights (same 16.8 MB bf16 SBUF footprint as
one full expert) and processes ALL of the pair's tokens at half F:

    per-core slots = ca_pad + cb_pad  (~4352 half-F token slots)
                   = ~2176 full-token equivalents vs 2304 for pure
                     expert-parallel -> ~6% less PE work.

GEMM2 contracts only the local F half, so each token's output is a
partial; the host sums the two cores' partials ("combine" still on
host).  Everything else (bf16, resident weights, chunk-major x, lag-1
GEMM2 pipeline, graded weight-slice streaming) matches the resident
kernel.
"""

import os
import sys
import numpy as np

for _p in ("/opt/trn_rl_repo", "/root/.axon_site/_ro/trn_rl_repo"):
    if _p not in sys.path and os.path.isdir(_p):
        sys.path.append(_p)

import concourse.bacc as bacc  # noqa: E402
import concourse.tile as tile  # noqa: E402
from concourse import mybir  # noqa: E402
from concourse.bass_utils import run_bass_kernel_spmd  # noqa: E402

# Problem shapes (hardcoded per spec)
B, S, H, F, E = 4, 2048, 1024, 4096, 8
T = B * S
TOP_K = 2
N_CORES = 8
P = 128
KH = H // P          # 8   H-contraction subtiles
F2 = F // 2          # 2048 F per core (half of one expert)
FT2 = F2 // P        # 16  f-tiles per expert half
CHUNK = 256          # tokens per GEMM1 chunk

F32 = mybir.dt.float32
BF16 = mybir.dt.bfloat16

_CACHE: dict = {}
LAST_RESULT = None  # BassKernelResults of the most recent run (for test.py)


def _build(ca_pad: int, cb_pad: int, use_b1: bool, mm_dt):
    nc = bacc.Bacc(
        "TRN2",
        target_bir_lowering=False,
        debug=False,
        enable_asserts=False,
        num_devices=N_CORES,
    )

    na = ca_pad // CHUNK
    nb = cb_pad // CHUNK
    n_chunks = na + nb
    c_pad = ca_pad + cb_pad
    NF = 2 * FT2  # 32 f-tiles total (16 per expert)

    xd = nc.dram_tensor(
        "xd", [P, n_chunks, KH, CHUNK], mm_dt, kind="ExternalInput"
    ).ap()
    w1d = nc.dram_tensor("w1d", [P, NF, KH, P], mm_dt, kind="ExternalInput").ap()
    w2d = nc.dram_tensor("w2d", [P, NF, H], mm_dt, kind="ExternalInput").ap()
    if use_b1:
        b1d = nc.dram_tensor("b1d", [P, NF], F32, kind="ExternalInput").ap()
    yd = nc.dram_tensor("yd", [P, c_pad // P, H], F32, kind="ExternalOutput").ap()

    gelu = mybir.ActivationFunctionType.Gelu_apprx_tanh

    with tile.TileContext(nc) as tc:
        with (
            tc.tile_pool(name="w1p", bufs=1) as w1p,
            tc.tile_pool(name="w2p", bufs=1) as w2p,
            tc.tile_pool(name="xp", bufs=3) as xp,
            tc.tile_pool(name="hp", bufs=3) as hp,
            tc.tile_pool(name="op", bufs=4) as op,
            tc.tile_pool(name="bp", bufs=1) as bp,
            tc.tile_pool(name="ps1", bufs=3, space="PSUM") as ps1,
            tc.tile_pool(name="ps2", bufs=4, space="PSUM") as ps2,
            tc.tile_pool(name="wup", bufs=1) as wup,
            tc.tile_pool(name="psw", bufs=1, space="PSUM") as psw,
        ):
            # PE warmup: dummy matmuls on a zeroed scratch tile fill the
            # initial DMA-wait window so the HAM clock gate reaches 8/8
            # (2.4 GHz) before the first real matmul (~3.4us of sustained
            # PE activity required).
            wt = wup.tile([P, 512], mm_dt)
            nc.gpsimd.memset(wt[:], 0.0)
            wu_ps = psw.tile([P, 512], F32)
            for _ in range(12):
                nc.tensor.matmul(wu_ps[:], wt[:, :P], wt[:], start=True, stop=True)

            if use_b1:
                b1t = bp.tile([P, NF], F32)
                nc.sync.dma_start(b1t[:], b1d[:])

            # weight stream order: w1(expert a) graded, w2(a), w1(b), w2(b)
            w1 = w1p.tile([P, NF, KH, P], mm_dt)
            w2 = w2p.tile([P, NF, H], mm_dt)

            def stream_w(eh):  # eh = 0 (expert a) or 1 (expert b)
                base = eh * FT2
                slices = (1, 1, 2, 4, 8) if eh == 0 else (8, 8)
                i = base
                for n in slices:
                    nc.scalar.dma_start(w1[:, i : i + n], w1d[:, i : i + n])
                    i += n
                for i in range(base, base + FT2, 8):
                    nc.scalar.dma_start(w2[:, i : i + 8], w2d[:, i : i + 8])

            stream_w(0)
            stream_w(1)

            hqs = [None] * n_chunks

            def gemm1(ci):
                f0 = 0 if ci < na else FT2
                xt = xp.tile([P, KH, CHUNK], mm_dt, tag="xt", name=f"xt_{ci}")
                nc.sync.dma_start(xt[:], xd[:, ci])
                hq = hp.tile([P, FT2, CHUNK], mm_dt, tag="hq", name=f"hq_{ci}")
                hqs[ci] = hq
                for fi in range(FT2):
                    f = f0 + fi
                    pt1 = ps1.tile([P, CHUNK], F32, tag="pt1")
                    for k in range(KH):
                        nc.tensor.matmul(
                            pt1[:],
                            w1[:, f, k, :],
                            xt[:, k, :],
                            start=(k == 0),
                            stop=(k == KH - 1),
                        )
                    bias = b1t[:, f : f + 1] if use_b1 else 0.0
                    nc.scalar.activation(hq[:, fi, :], pt1[:], gelu, bias=bias)

            def gemm2(ci):
                f0 = 0 if ci < na else FT2
                hq = hqs[ci]
                for t in range(CHUNK // P):
                    trow = ci * (CHUNK // P) + t
                    pts = [
                        ps2.tile([P, 512], F32, tag="pt2", name=f"pt2_{hh}")
                        for hh in range(2)
                    ]
                    for k2 in range(FT2):
                        for hh in range(2):
                            nc.tensor.matmul(
                                pts[hh][:],
                                hq[:, k2, t * P : (t + 1) * P],
                                w2[:, f0 + k2, hh * 512 : (hh + 1) * 512],
                                start=(k2 == 0),
                                stop=(k2 == FT2 - 1),
                            )
                    for hh in range(2):
                        ot = op.tile([P, 512], F32, tag="ot")
                        nc.vector.tensor_copy(ot[:], pts[hh][:])
                        nc.sync.dma_start(
                            yd[:, trow, hh * 512 : (hh + 1) * 512], ot[:]
                        )
                hqs[ci] = None

            for ci in range(n_chunks):
                gemm1(ci)
                if ci >= 1:
                    gemm2(ci - 1)
            gemm2(n_chunks - 1)

    nc.compile()
    return nc


def _route(x2d, Wg):
    """Replicates reference router: softmax -> top-2 -> renormalize."""
    logits = x2d @ Wg  # [T, E] fp32
    m = logits.max(axis=-1, keepdims=True)
    p = np.exp(logits - m, dtype=np.float32)
    p /= p.sum(axis=-1, keepdims=True)
    # jax.lax.top_k: values descending, ties broken by lower index.
    order = np.argsort(-p, axis=-1, kind="stable")
    top_i = order[:, :TOP_K]  # [T, 2]
    top_p = np.take_along_axis(p, top_i, axis=-1)
    top_p = top_p / top_p.sum(axis=-1, keepdims=True)
    return top_i, top_p


def _pad_chunks(c: int) -> int:
    return max(CHUNK, ((c + CHUNK - 1) // CHUNK) * CHUNK)


def kernel(x, Wg, W1, b1, W2, b2):
    global LAST_RESULT
    x = np.ascontiguousarray(np.asarray(x, dtype=np.float32))
    Wg = np.ascontiguousarray(np.asarray(Wg, dtype=np.float32))
    W1 = np.ascontiguousarray(np.asarray(W1, dtype=np.float32))
    b1 = np.ascontiguousarray(np.asarray(b1, dtype=np.float32))
    W2 = np.ascontiguousarray(np.asarray(W2, dtype=np.float32))
    b2 = np.ascontiguousarray(np.asarray(b2, dtype=np.float32))

    x2d = x.reshape(T, H)
    top_i, top_p = _route(x2d, Wg)

    rows = [None] * E
    gval = [None] * E
    for e in range(E):
        r, slot = np.nonzero(top_i == e)
        rows[e] = r
        gval[e] = top_p[r, slot]

    counts = np.array([len(r) for r in rows])
    order = np.argsort(-counts, kind="stable")
    pairs = [(int(order[i]), int(order[E - 1 - i])) for i in range(E // 2)]
    ca_pad = _pad_chunks(int(counts[[p[0] for p in pairs]].max()))
    cb_pad = _pad_chunks(int(counts[[p[1] for p in pairs]].max()))
    use_b1 = bool(np.any(b1))

    mm_dt = {
        "bf16": BF16,
        "fp32": F32,
    }[os.environ.get("KERNEL_MMDT", "bf16")]
    key = (ca_pad, cb_pad, use_b1, str(mm_dt))
    if key not in _CACHE:
        _CACHE[key] = _build(ca_pad, cb_pad, use_b1, mm_dt)
    nc = _CACHE[key]

    np_dt = mybir.dt.np(mm_dt)
    c_pad = ca_pad + cb_pad
    n_chunks = c_pad // CHUNK

    def pack_w(e, h):
        lo, hi = h * F2, (h + 1) * F2
        w1p = np.ascontiguousarray(
            W1[e][:, lo:hi].reshape(KH, P, FT2, P).transpose(1, 2, 0, 3)
        )
        w2p = np.ascontiguousarray(
            W2[e][lo:hi, :].reshape(FT2, P, H).transpose(1, 0, 2)
        )
        return w1p.astype(np_dt), w2p.astype(np_dt)

    in_maps = [None] * N_CORES
    for pi, (a, b) in enumerate(pairs):
        xt = np.zeros((H, c_pad), np.float32)
        xt[:, : counts[a]] = x2d[rows[a]].T
        xt[:, ca_pad : ca_pad + counts[b]] = x2d[rows[b]].T
        xd = np.ascontiguousarray(
            xt.reshape(KH, P, n_chunks, CHUNK).transpose(1, 2, 0, 3).astype(np_dt)
        )
        for h in range(2):
            w1a, w2a = pack_w(a, h)
            w1b, w2b = pack_w(b, h)
            m = {
                "xd": xd,
                "w1d": np.ascontiguousarray(np.concatenate([w1a, w1b], axis=1)),
                "w2d": np.ascontiguousarray(np.concatenate([w2a, w2b], axis=1)),
            }
            if use_b1:
                lo, hi = h * F2, (h + 1) * F2
                m["b1d"] = np.ascontiguousarray(
                    np.concatenate(
                        [
                            b1[a][lo:hi].reshape(FT2, P).T,
                            b1[b][lo:hi].reshape(FT2, P).T,
                        ],
                        axis=1,
                    )
                )
            in_maps[2 * pi + h] = m

    trace = os.environ.get("KERNEL_TRACE", "") == "1"
    res = run_bass_kernel_spmd(
        nc,
        in_maps,
        core_ids=list(range(N_CORES)),
        trace=trace,
        trace_cores=[0] if trace else None,
    )
    LAST_RESULT = res

    out = np.zeros((T, H), np.float32)
    for pi, (a, b) in enumerate(pairs):
        y = res.results[2 * pi]["yd"] + res.results[2 * pi + 1]["yd"]
        y = y.transpose(1, 0, 2).reshape(c_pad, H)
        out[rows[a]] += gval[a][:, None] * (y[: counts[a]] + b2[a][None, :])
        out[rows[b]] += gval[b][:, None] * (
            y[ca_pad : ca_pad + counts[b]] + b2[b][None, :]
        )

    return out.reshape(B, S, H)


# revision 16
# speedup vs baseline: 1.1589x; 1.0010x over previous
"""MoE FFN (top-2 of 8 experts) Trainium2 kernel, pair/F-split variant.

Same host-side routing as the resident-weight kernel, but load-balanced:
experts are sorted by token count and paired heavy-with-light; pair p is
assigned to cores (2p, 2p+1).  Each core of the pair holds HALF the F
dimension of BOTH experts' weights (same 16.8 MB bf16 SBUF footprint as
one full expert) and processes ALL of the pair's tokens at half F:

    per-core slots = ca_pad + cb_pad  (~4352 half-F token slots)
                   = ~2176 full-token equivalents vs 2304 for pure
                     expert-parallel -> ~6% less PE work.

GEMM2 contracts only the local F half, so each token's output is a
partial; the host sums the two cores' partials ("combine" still on
host).  Everything else (bf16, resident weights, chunk-major x, lag-1
GEMM2 pipeline, graded weight-slice streaming) matches the resident
kernel.
"""

import os
import sys
import numpy as np

for _p in ("/opt/trn_rl_repo", "/root/.axon_site/_ro/trn_rl_repo"):
    if _p not in sys.path and os.path.isdir(_p):
        sys.path.append(_p)

import concourse.bacc as bacc  # noqa: E402
import concourse.tile as tile  # noqa: E402
from concourse import mybir  # noqa: E402
from concourse.bass_utils import run_bass_kernel_spmd  # noqa: E402

# Problem shapes (hardcoded per spec)
B, S, H, F, E = 4, 2048, 1024, 4096, 8
T = B * S
TOP_K = 2
N_CORES = 8
P = 128
KH = H // P          # 8   H-contraction subtiles
F2 = F // 2          # 2048 F per core (half of one expert)
FT2 = F2 // P        # 16  f-tiles per expert half
CHUNK = 256          # tokens per GEMM1 chunk

F32 = mybir.dt.float32
BF16 = mybir.dt.bfloat16

_CACHE: dict = {}
LAST_RESULT = None  # BassKernelResults of the most recent run (for test.py)


def _build(ca_pad: int, cb_pad: int, use_b1: bool, mm_dt):
    nc = bacc.Bacc(
        "TRN2",
        target_bir_lowering=False,
        debug=False,
        enable_asserts=False,
        num_devices=N_CORES,
    )

    c_pad = ca_pad + cb_pad
    NF = 2 * FT2  # 32 f-tiles total (16 per expert)
    # mixed 512/256 token chunks (512 halves GEMM1 instruction count)
    chunks = []  # (token_offset, nt, f0)
    for base, cp, f0 in ((0, ca_pad, 0), (ca_pad, cb_pad, FT2)):
        off = base
        for nt in [512] * (cp // 512) + ([256] if cp % 512 else []):
            chunks.append((off, nt, f0))
            off += nt
    n_chunks = len(chunks)

    # flat x layout: each chunk is a contiguous [KH, nt] block per partition
    xd = nc.dram_tensor(
        "xd", [P, KH * c_pad], mm_dt, kind="ExternalInput"
    ).ap()
    w1d = nc.dram_tensor("w1d", [P, NF, KH, P], mm_dt, kind="ExternalInput").ap()
    w2d = nc.dram_tensor("w2d", [P, NF, H], mm_dt, kind="ExternalInput").ap()
    if use_b1:
        b1d = nc.dram_tensor("b1d", [P, NF], F32, kind="ExternalInput").ap()
    yd = nc.dram_tensor("yd", [P, c_pad // P, H], F32, kind="ExternalOutput").ap()

    gelu = mybir.ActivationFunctionType.Gelu_apprx_tanh

    with tile.TileContext(nc) as tc:
        with (
            tc.tile_pool(name="w1p", bufs=1) as w1p,
            tc.tile_pool(name="w2p", bufs=1) as w2p,
            tc.tile_pool(name="xp", bufs=2) as xp,
            tc.tile_pool(name="hp", bufs=3) as hp,
            tc.tile_pool(name="op", bufs=4) as op,
            tc.tile_pool(name="bp", bufs=1) as bp,
            tc.tile_pool(name="ps1", bufs=3, space="PSUM") as ps1,
            tc.tile_pool(name="ps2", bufs=4, space="PSUM") as ps2,
            tc.tile_pool(name="wup", bufs=1) as wup,
            tc.tile_pool(name="psw", bufs=1, space="PSUM") as psw,
        ):
            # PE warmup: dummy matmuls on a zeroed scratch tile fill the
            # initial DMA-wait window so the HAM clock gate reaches 8/8
            # (2.4 GHz) before the first real matmul (~3.4us of sustained
            # PE activity required).
            wt = wup.tile([P, 512], mm_dt)
            nc.gpsimd.memset(wt[:], 0.0)
            wu_ps = psw.tile([P, 512], F32)
            for _ in range(26):
                nc.tensor.matmul(wu_ps[:], wt[:, :P], wt[:], start=True, stop=True)

            if use_b1:
                b1t = bp.tile([P, NF], F32)
                nc.sync.dma_start(b1t[:], b1d[:])

            # weight stream order: w1(expert a) graded, w2(a), w1(b), w2(b)
            w1 = w1p.tile([P, NF, KH, P], mm_dt)
            w2 = w2p.tile([P, NF, H], mm_dt)

            def stream_w(eh):  # eh = 0 (expert a) or 1 (expert b)
                base = eh * FT2
                slices = (1, 1, 2, 4, 8) if eh == 0 else (8, 8)
                i = base
                for n in slices:
                    nc.scalar.dma_start(w1[:, i : i + n], w1d[:, i : i + n])
                    i += n
                for i in range(base, base + FT2, 8):
                    nc.scalar.dma_start(w2[:, i : i + 8], w2d[:, i : i + 8])

            stream_w(0)
            stream_w(1)

            hqs = [None] * n_chunks

            def gemm1(ci):
                off, nt, f0 = chunks[ci]
                xt = xp.tile([P, KH * 512], mm_dt, tag="xt", name=f"xt_{ci}")
                nc.sync.dma_start(
                    xt[:, : KH * nt], xd[:, KH * off : KH * (off + nt)]
                )
                hq = hp.tile([P, FT2, 512], mm_dt, tag="hq", name=f"hq_{ci}")
                hqs[ci] = hq
                for fi in range(FT2):
                    f = f0 + fi
                    pt1 = ps1.tile([P, 512], F32, tag="pt1")
                    for k in range(KH):
                        nc.tensor.matmul(
                            pt1[:, :nt],
                            w1[:, f, k, :],
                            xt[:, k * nt : (k + 1) * nt],
                            start=(k == 0),
                            stop=(k == KH - 1),
                        )
                    bias = b1t[:, f : f + 1] if use_b1 else 0.0
                    nc.scalar.activation(hq[:, fi, :nt], pt1[:, :nt], gelu, bias=bias)

            def gemm2(ci):
                off, nt, f0 = chunks[ci]
                hq = hqs[ci]
                for t in range(nt // P):
                    trow = off // P + t
                    last = ci == n_chunks - 1 and t == nt // P - 1
                    pts = [
                        ps2.tile([P, 512], F32, tag="pt2", name=f"pt2_{hh}")
                        for hh in range(2)
                    ]
                    for k2 in range(FT2):
                        for hh in range(2):
                            nc.tensor.matmul(
                                pts[hh][:],
                                hq[:, k2, t * P : (t + 1) * P],
                                w2[:, f0 + k2, hh * 512 : (hh + 1) * 512],
                                start=(k2 == 0),
                                stop=(k2 == FT2 - 1),
                            )
                    # narrower copy/DMA strips on the final tile shorten the
                    # post-last-matmul drain before the end-of-kernel barrier
                    strips = 2 if last else 1
                    for hh in range(2):
                        for q in range(strips):
                            w = 512 // strips
                            ot = op.tile([P, 512], F32, tag="ot", padded_shape=[P, 512])
                            nc.vector.tensor_copy(
                                ot[:, : w], pts[hh][:, q * w : (q + 1) * w]
                            )
                            nc.sync.dma_start(
                                yd[:, trow, hh * 512 + q * w : hh * 512 + (q + 1) * w],
                                ot[:, : w],
                            )
                hqs[ci] = None

            for ci in range(n_chunks):
                gemm1(ci)
                if ci >= 1:
                    gemm2(ci - 1)
            gemm2(n_chunks - 1)

    nc.compile()
    return nc


def _route(x2d, Wg):
    """Replicates reference router: softmax -> top-2 -> renormalize."""
    logits = x2d @ Wg  # [T, E] fp32
    m = logits.max(axis=-1, keepdims=True)
    p = np.exp(logits - m, dtype=np.float32)
    p /= p.sum(axis=-1, keepdims=True)
    # jax.lax.top_k: values descending, ties broken by lower index.
    order = np.argsort(-p, axis=-1, kind="stable")
    top_i = order[:, :TOP_K]  # [T, 2]
    top_p = np.take_along_axis(p, top_i, axis=-1)
    top_p = top_p / top_p.sum(axis=-1, keepdims=True)
    return top_i, top_p


def _pad_chunks(c: int) -> int:
    return max(CHUNK, ((c + CHUNK - 1) // CHUNK) * CHUNK)


def kernel(x, Wg, W1, b1, W2, b2):
    global LAST_RESULT
    x = np.ascontiguousarray(np.asarray(x, dtype=np.float32))
    Wg = np.ascontiguousarray(np.asarray(Wg, dtype=np.float32))
    W1 = np.ascontiguousarray(np.asarray(W1, dtype=np.float32))
    b1 = np.ascontiguousarray(np.asarray(b1, dtype=np.float32))
    W2 = np.ascontiguousarray(np.asarray(W2, dtype=np.float32))
    b2 = np.ascontiguousarray(np.asarray(b2, dtype=np.float32))

    x2d = x.reshape(T, H)
    top_i, top_p = _route(x2d, Wg)

    rows = [None] * E
    gval = [None] * E
    for e in range(E):
        r, slot = np.nonzero(top_i == e)
        rows[e] = r
        gval[e] = top_p[r, slot]

    counts = np.array([len(r) for r in rows])
    order = np.argsort(-counts, kind="stable")
    pairs = [(int(order[i]), int(order[E - 1 - i])) for i in range(E // 2)]
    ca_pad = _pad_chunks(int(counts[[p[0] for p in pairs]].max()))
    cb_pad = _pad_chunks(int(counts[[p[1] for p in pairs]].max()))
    use_b1 = bool(np.any(b1))

    mm_dt = {
        "bf16": BF16,
        "fp32": F32,
    }[os.environ.get("KERNEL_MMDT", "bf16")]
    key = (ca_pad, cb_pad, use_b1, str(mm_dt))
    if key not in _CACHE:
        _CACHE[key] = _build(ca_pad, cb_pad, use_b1, mm_dt)
    nc = _CACHE[key]

    np_dt = mybir.dt.np(mm_dt)
    c_pad = ca_pad + cb_pad
    chunk_sizes = []
    for cp in (ca_pad, cb_pad):
        chunk_sizes += [512] * (cp // 512) + ([256] if cp % 512 else [])

    def pack_w(e, h):
        lo, hi = h * F2, (h + 1) * F2
        w1p = np.ascontiguousarray(
            W1[e][:, lo:hi].reshape(KH, P, FT2, P).transpose(1, 2, 0, 3)
        )
        w2p = np.ascontiguousarray(
            W2[e][lo:hi, :].reshape(FT2, P, H).transpose(1, 0, 2)
        )
        return w1p.astype(np_dt), w2p.astype(np_dt)

    in_maps = [None] * N_CORES
    for pi, (a, b) in enumerate(pairs):
        xt = np.zeros((H, c_pad), np.float32)
        xt[:, : counts[a]] = x2d[rows[a]].T
        xt[:, ca_pad : ca_pad + counts[b]] = x2d[rows[b]].T
        xt = xt.reshape(KH, P, c_pad).astype(np_dt)
        parts, off = [], 0
        for nt in chunk_sizes:
            parts.append(
                xt[:, :, off : off + nt].transpose(1, 0, 2).reshape(P, KH * nt)
            )
            off += nt
        xd = np.ascontiguousarray(np.concatenate(parts, axis=1))
        for h in range(2):
            w1a, w2a = pack_w(a, h)
            w1b, w2b = pack_w(b, h)
            m = {
                "xd": xd,
                "w1d": np.ascontiguousarray(np.concatenate([w1a, w1b], axis=1)),
                "w2d": np.ascontiguousarray(np.concatenate([w2a, w2b], axis=1)),
            }
            if use_b1:
                lo, hi = h * F2, (h + 1) * F2
                m["b1d"] = np.ascontiguousarray(
                    np.concatenate(
                        [
                            b1[a][lo:hi].reshape(FT2, P).T,
                            b1[b][lo:hi].reshape(FT2, P).T,
                        ],
                        axis=1,
                    )
                )
            in_maps[2 * pi + h] = m

    trace = os.environ.get("KERNEL_TRACE", "") == "1"
    res = run_bass_kernel_spmd(
        nc,
        in_maps,
        core_ids=list(range(N_CORES)),
        trace=trace,
        trace_cores=[0] if trace else None,
    )
    LAST_RESULT = res

    out = np.zeros((T, H), np.float32)
    for pi, (a, b) in enumerate(pairs):
        y = res.results[2 * pi]["yd"] + res.results[2 * pi + 1]["yd"]
        y = y.transpose(1, 0, 2).reshape(c_pad, H)
        out[rows[a]] += gval[a][:, None] * (y[: counts[a]] + b2[a][None, :])
        out[rows[b]] += gval[b][:, None] * (
            y[ca_pad : ca_pad + counts[b]] + b2[b][None, :]
        )

    return out.reshape(B, S, H)
